# revision 10
# baseline (speedup 1.0000x reference)
"""Trainium2 Bass kernel for nn_MoELayerStacks (moe_routing).

Computation (see reference):
  router:  gate_logits = router_input @ router_w.T + router_b  -> softmax,
           argmax one-hot, aux/z losses, entropy, top1 (batch means).
  l1:      [B,3072] @ [3072,128] (merged factorized weights), bias,
           split 15 l1x + 1 l1x_out per expert,
           h = clip([sq(l1x)*255/256, l1x], 0, 1).
  l2:      per-expert [30 -> 32], clip to [0,1].
  l3:      per-expert [32 -> 1] + l1x_out;  out[b] = l3x[b, argmax[b]].

Sharding: data-parallel over batch across 8 cores (2048 rows each), tiny
weights replicated; scalar/batch-mean reductions finished on host.

Device layout notes:
  - Host pre-transposes activations (xT [D, Bc], rT [R, Bc]) so the
    contraction dim lands on SBUF partitions; everything on-device runs in
    "feature-on-partition, batch-on-free" orientation until a final PE
    transpose of the tiny [8, B] l3x tile.
  - l1 output features are permuted so each expert-group's l2 inputs form a
    32-aligned partition window (matmul operand base partitions must be in
    {0, 32, 64, 96}): rows 0:60 = experts 0-3 l1x, 60:64 = l1x_out e0-3,
    64:124 = experts 4-7 l1x, 124:128 = l1x_out e4-7.
  - l2/l3 use block-diagonal weights (4 experts per matmul); the l1x_out
    residual is folded into the l3 PSUM accumulation via selector matmuls.
  - fp32 matmuls are bitcast to float32r (full PE rate at N>=256).
  - Router stats (mask/probs/z/entropy/top1 row values) are reduced over
    the batch with ones-vector matmuls into a [1, 304] stats tensor.
"""

import math
from contextlib import ExitStack

import numpy as np
import ml_dtypes

import concourse.bass as bass
import concourse.mybir as mybir
import concourse.tile as tile
from concourse import bass_utils
from concourse.masks import make_identity

# ---- problem constants (hardcoded per contract) ----
E = 8
L2 = 15
L3 = 32
D = 3072
R = 256
B = 16384
AUX_ALPHA = 0.01
Z_ALPHA = 0.001

NCORES = 8
BC = B // NCORES          # 2048 rows per core
TN = 512                  # batch tile (PSUM bank = 512 fp32)
NT = BC // TN             # 4 batch tiles per core
NS = BC // 128            # 16 subtiles of 128 rows
KC = D // 128             # 24 contraction chunks

F32 = mybir.dt.float32
F32R = mybir.dt.float32r
BF16 = mybir.dt.bfloat16

XDT = BF16                # dtype for x / l1 weights (DMA-dominant tensors)
SQ_SCALE = math.sqrt(255.0 / 256.0)


def _r(ap):
    """Bitcast an fp32 AP to float32r for full-rate PE streaming."""
    return ap.bitcast(F32R)


def split_sync_waits(nc, max_waits=1):
    """This walrus build caps sync waits per instruction (CTRL drain fails
    with >1); hoist extras onto preceding same-engine NoOps."""
    n = 0
    for f in nc.m.functions:
        for blk in f.blocks:
            new_insts = []
            for inst in blk.instructions:
                si = inst.sync_info
                if si is not None and si.on_wait and len(si.on_wait) > max_waits:
                    waits = list(si.on_wait)
                    extra, keep = waits[:-max_waits], waits[-max_waits:]
                    for i in range(0, len(extra), max_waits):
                        n += 1
                        nop = mybir.InstNoOp(name=f"W-split-{n}", ins=[], outs=[])
                        nop.engine = inst.engine
                        nop.sync_info = mybir.SyncInfo(
                            on_wait=extra[i : i + max_waits], on_update=[]
                        )
                        new_insts.append(nop)
                    inst.sync_info = mybir.SyncInfo(
                        on_wait=keep, on_update=list(si.on_update or [])
                    )
                new_insts.append(inst)
            blk.instructions = new_insts
    return n


def build_nc(reps: int = 1, split: bool = True) -> bass.Bass:
    nc = bass.Bass(target_bir_lowering=True)

    # ---- per-core DRAM tensors ----
    xT = nc.dram_tensor("xT", [D, BC], XDT, kind="ExternalInput")
    rT = nc.dram_tensor("rT", [R, BC], F32, kind="ExternalInput")
    wT = nc.dram_tensor("wT", [D, 128], XDT, kind="ExternalInput")
    bvec = nc.dram_tensor("bvec", [128, 1], F32, kind="ExternalInput")
    lq = nc.dram_tensor("lq", [128, 256], F32, kind="ExternalInput")
    lr = nc.dram_tensor("lr", [128, 256], F32, kind="ExternalInput")
    zbias = nc.dram_tensor("zbias", [128, 2], F32, kind="ExternalInput")
    lo = nc.dram_tensor("lo", [128, 16], F32, kind="ExternalInput")
    sel = nc.dram_tensor("sel", [128, 16], F32, kind="ExternalInput")
    obias = nc.dram_tensor("obias", [8, 1], F32, kind="ExternalInput")
    rwT = nc.dram_tensor("rwT", [R, E], F32, kind="ExternalInput")
    rbrow = nc.dram_tensor("rbrow", [1, NS * E], F32, kind="ExternalInput")
    onesrow = nc.dram_tensor("onesrow", [1, 128], F32, kind="ExternalInput")
    onescol = nc.dram_tensor("onescol", [128, 1], F32, kind="ExternalInput")

    out_col = nc.dram_tensor("out_col", [128, NS], F32, kind="ExternalOutput")
    stats = nc.dram_tensor("stats", [1, 304], F32, kind="ExternalOutput")

    with ExitStack() as ctx:
        tc = ctx.enter_context(tile.TileContext(nc))
        cpool = ctx.enter_context(tc.tile_pool(name="consts", bufs=1))

        # ---- resident constants ----
        wT_sb = cpool.tile([128, KC * 128], XDT)       # chunk k at cols k*128
        for k in range(KC):
            nc.sync.dma_start(
                wT_sb[:, k * 128 : (k + 1) * 128], wT[k * 128 : (k + 1) * 128, :]
            )
        rT_sb = cpool.tile([128, 2 * BC], F32)         # chunk k at cols k*BC
        for k in range(2):
            nc.sync.dma_start(
                rT_sb[:, k * BC : (k + 1) * BC], rT[k * 128 : (k + 1) * 128, :]
            )
        rw_sb = cpool.tile([128, 2 * E], F32)          # chunk k at cols k*8
        for k in range(2):
            nc.sync.dma_start(
                rw_sb[:, k * E : (k + 1) * E], rwT[k * 128 : (k + 1) * 128, :]
            )
        bvec_sb = cpool.tile([128, 1], F32)
        nc.sync.dma_start(bvec_sb[:], bvec[:])
        lq_sb = cpool.tile([128, 256], F32)
        nc.sync.dma_start(lq_sb[:], lq[:])
        lr_sb = cpool.tile([128, 256], F32)
        nc.sync.dma_start(lr_sb[:], lr[:])
        zb_sb = cpool.tile([128, 2], F32)
        nc.sync.dma_start(zb_sb[:], zbias[:])
        lo_sb = cpool.tile([128, 16], F32)
        nc.sync.dma_start(lo_sb[:], lo[:])
        sel_sb = cpool.tile([128, 16], F32)
        nc.sync.dma_start(sel_sb[:], sel[:])
        ob_sb = cpool.tile([8, 1], F32)
        nc.sync.dma_start(ob_sb[:], obias[:])
        rb_sb = cpool.tile([1, NS * E], F32)
        nc.sync.dma_start(rb_sb[:], rbrow[:])
        or_sb = cpool.tile([1, 128], F32)
        nc.sync.dma_start(or_sb[:], onesrow[:])
        oc_sb = cpool.tile([128, 1], F32)
        nc.sync.dma_start(oc_sb[:], onescol[:])
        ident = cpool.tile([128, 128], F32)
        make_identity(nc, ident[:])

        # f32r-rounded copies of the l2/l3 weights (walrus requires f32r
        # matmul inputs to be produced by a rounding instruction)
        lq_r = cpool.tile([128, 256], F32R)
        nc.vector.tensor_copy(lq_r[:], lq_sb[:])
        lr_r = cpool.tile([128, 256], F32R)
        nc.vector.tensor_copy(lr_r[:], lr_sb[:])
        lo_r = cpool.tile([128, 16], F32R)
        nc.vector.tensor_copy(lo_r[:], lo_sb[:])
        sel_r = cpool.tile([128, 16], F32R)
        nc.vector.tensor_copy(sel_r[:], sel_sb[:])

        # persistent work tiles (whole-core router state, output staging)
        mask_sb = cpool.tile([128, NS * E], F32)
        out_sb = cpool.tile([128, NS], F32)

        for _ in range(reps):
            with (
                ExitStack() as rctx,
                tc.tile_pool(name="xp", bufs=2) as xpool,
                tc.tile_pool(name="rtr", bufs=1) as rpool,
                tc.tile_pool(name="work", bufs=2) as wpool,
            ):
                # issue first x slab load before router work so it overlaps
                x_sb = xpool.tile([128, KC * TN], XDT, tag="xslab")
                nc.sync.dma_start(
                    x_sb[:].rearrange("p (k n) -> p k n", n=TN),
                    xT[:]
                    .rearrange("(k p) b -> p k b", p=128)[:, :, 0:TN],
                )
                slabs = [x_sb]

                # ---- phase 1: router for the whole core ----
                phase1_cm = tc.tile_pool(name="ps_r", bufs=1, space="PSUM")
                ps_r = phase1_cm.__enter__()
                gl_ps = ps_r.tile([128, NS * E], F32)
                for s in range(NS):
                    for k in range(2):
                        nc.tensor.matmul(
                            gl_ps[:, s * E : (s + 1) * E],
                            rT_sb[:, k * BC + s * 128 : k * BC + (s + 1) * 128],
                            rw_sb[:, k * E : (k + 1) * E],
                            start=(k == 0),
                            stop=False,
                        )
                    nc.tensor.matmul(
                        gl_ps[:, s * E : (s + 1) * E],
                        or_sb[0:1, :],
                        rb_sb[0:1, s * E : (s + 1) * E],
                        start=False,
                        stop=True,
                    )
                gl_sb = rpool.tile([128, NS * E], F32)
                nc.scalar.copy(gl_sb[:], gl_ps[:])
                gl3 = gl_sb[:].rearrange("p (s e) -> p s e", e=E)

                m_sb = rpool.tile([128, NS], F32)
                nc.vector.reduce_max(m_sb[:], gl3, axis=mybir.AxisListType.X)
                mb = m_sb[:, :, None].broadcast_to([128, NS, E])

                t_sb = rpool.tile([128, NS * E], F32)
                t3 = t_sb[:].rearrange("p (s e) -> p s e", e=E)
                nc.vector.tensor_tensor(t3, gl3, mb, op=mybir.AluOpType.subtract)

                e_sb = rpool.tile([128, NS * E], F32)
                nc.scalar.activation(e_sb[:], t_sb[:], mybir.ActivationFunctionType.Exp)
                e3 = e_sb[:].rearrange("p (s e) -> p s e", e=E)

                ssum = rpool.tile([128, NS], F32)
                nc.vector.reduce_sum(ssum[:], e3, axis=mybir.AxisListType.X)
                rec = rpool.tile([128, NS], F32)
                nc.vector.reciprocal(rec[:], ssum[:])
                recb = rec[:, :, None].broadcast_to([128, NS, E])

                probs = rpool.tile([128, NS * E], F32)
                p3 = probs[:].rearrange("p (s e) -> p s e", e=E)
                nc.vector.tensor_tensor(p3, e3, recb, op=mybir.AluOpType.mult)

                m3 = mask_sb[:].rearrange("p (s e) -> p s e", e=E)
                nc.vector.tensor_tensor(m3, gl3, mb, op=mybir.AluOpType.is_equal)

                lns = rpool.tile([128, NS], F32)
                nc.scalar.activation(lns[:], ssum[:], mybir.ActivationFunctionType.Ln)
                lse = rpool.tile([128, NS], F32)
                nc.vector.tensor_add(lse[:], m_sb[:], lns[:])
                zrow = rpool.tile([128, NS], F32)
                nc.vector.tensor_mul(zrow[:], lse[:], lse[:])

                et_sb = rpool.tile([128, NS * E], F32)
                et3 = et_sb[:].rearrange("p (s e) -> p s e", e=E)
                nc.vector.tensor_tensor(et3, e3, t3, op=mybir.AluOpType.mult)
                u_sb = rpool.tile([128, NS], F32)
                nc.vector.reduce_sum(u_sb[:], et3, axis=mybir.AxisListType.X)
                v_sb = rpool.tile([128, NS], F32)
                nc.vector.tensor_mul(v_sb[:], u_sb[:], rec[:])
                entrow = rpool.tile([128, NS], F32)
                nc.vector.tensor_sub(entrow[:], lns[:], v_sb[:])

                stats_ps = ps_r.tile([1, 304], F32)
                for rhs, off, w in (
                    (mask_sb, 0, 128),
                    (probs, 128, 128),
                    (zrow, 256, NS),
                    (entrow, 272, NS),
                    (rec, 288, NS),
                ):
                    nc.tensor.matmul(
                        stats_ps[0:1, off : off + w],
                        oc_sb[:, 0:1],
                        rhs[:, 0:w],
                        start=True,
                        stop=True,
                    )
                stats_sb = rpool.tile([1, 304], F32)
                nc.scalar.copy(stats_sb[:], stats_ps[:])
                nc.sync.dma_start(stats[:], stats_sb[:])
                phase1_cm.__exit__(None, None, None)

                # ---- phase 2: l1/l2/l3 over 4 batch tiles ----
                ps1 = rctx.enter_context(
                    tc.tile_pool(name="ps1", bufs=2, space="PSUM")
                )
                psz = rctx.enter_context(
                    tc.tile_pool(name="psz", bufs=2, space="PSUM")
                )
                ps3 = rctx.enter_context(
                    tc.tile_pool(name="ps3", bufs=2, space="PSUM")
                )
                pst = rctx.enter_context(
                    tc.tile_pool(name="pst", bufs=2, space="PSUM")
                )
                for t in range(NT):
                    if t + 1 < NT:
                        x_nx = xpool.tile([128, KC * TN], XDT, tag="xslab")
                        nc.sync.dma_start(
                            x_nx[:].rearrange("p (k n) -> p k n", n=TN),
                            xT[:]
                            .rearrange("(k p) b -> p k b", p=128)[
                                :, :, (t + 1) * TN : (t + 2) * TN
                            ],
                        )
                        slabs.append(x_nx)
                    x_cur = slabs[t]

                    l1_ps = ps1.tile([128, TN], F32, tag="l1")
                    for k in range(KC):
                        nc.tensor.matmul(
                            l1_ps[:],
                            wT_sb[:, k * 128 : (k + 1) * 128],
                            x_cur[:, k * TN : (k + 1) * TN],
                            start=(k == 0),
                            stop=(k == KC - 1),
                        )
                    l1c = wpool.tile([128, TN], F32R, tag="l1c")
                    nc.scalar.activation(
                        l1c[:],
                        l1_ps[:],
                        mybir.ActivationFunctionType.Identity,
                        bias=bvec_sb[:, 0:1],
                    )
                    # h = [clip(sq(l1x)*255/256, 0, 1), clip(l1x, 0, 1)]
                    # (rows 60:64 / 124:128 are l1x_out; squared junk unused)
                    q_sb = wpool.tile([124, TN], F32R, tag="q")
                    nc.scalar.activation(
                        q_sb[0:124, :],
                        l1c[0:124, :],
                        mybir.ActivationFunctionType.Square,
                        scale=SQ_SCALE,
                    )
                    nc.vector.tensor_scalar_min(q_sb[0:124, :], q_sb[0:124, :], 1.0)
                    r_sb = wpool.tile([124, TN], F32R, tag="r")
                    nc.vector.tensor_scalar(
                        r_sb[0:124, :],
                        l1c[0:124, :],
                        0.0,
                        1.0,
                        op0=mybir.AluOpType.max,
                        op1=mybir.AluOpType.min,
                    )

                    l3_ps = ps3.tile([8, TN], F32, tag="l3")
                    for g in range(2):
                        base = 64 * g
                        z_ps = psz.tile([128, TN], F32, tag="z")
                        nc.tensor.matmul(
                            z_ps[:],
                            lq_r[base : base + 60, g * 128 : (g + 1) * 128],
                            q_sb[base : base + 60, :],
                            start=True,
                            stop=False,
                        )
                        nc.tensor.matmul(
                            z_ps[:],
                            lr_r[base : base + 60, g * 128 : (g + 1) * 128],
                            r_sb[base : base + 60, :],
                            start=False,
                            stop=True,
                        )
                        z_sb = wpool.tile([128, TN], F32R, tag="zsb")
                        nc.scalar.activation(
                            z_sb[:],
                            z_ps[:],
                            mybir.ActivationFunctionType.Identity,
                            bias=zb_sb[:, g : g + 1],
                        )
                        nc.vector.tensor_scalar(
                            z_sb[:],
                            z_sb[:],
                            0.0,
                            1.0,
                            op0=mybir.AluOpType.max,
                            op1=mybir.AluOpType.min,
                        )
                        nc.tensor.matmul(
                            l3_ps[:],
                            lo_r[:, g * 8 : (g + 1) * 8],
                            z_sb[:],
                            start=(g == 0),
                            stop=False,
                        )
                        # fold l1x_out residual: selector over 32-aligned window
                        nc.tensor.matmul(
                            l3_ps[:],
                            sel_r[32 + 64 * g : 64 + 64 * g, g * 8 : (g + 1) * 8],
                            l1c[32 + 64 * g : 64 + 64 * g, :],
                            start=False,
                            stop=(g == 1),
                            tile_position=(32 + 64 * g, 0),
                        )
                    l3x = wpool.tile([8, TN], F32, tag="l3x")
                    nc.scalar.activation(
                        l3x[:],
                        l3_ps[:],
                        mybir.ActivationFunctionType.Identity,
                        bias=ob_sb[:, 0:1],
                    )
                    l3t_ps = pst.tile([128, 4 * E], F32, tag="l3t")
                    for u in range(4):
                        nc.tensor.transpose(
                            l3t_ps[:, u * E : (u + 1) * E],
                            l3x[:, u * 128 : (u + 1) * 128],
                            ident[0:8, 0:8],
                        )
                    scratch = wpool.tile([128, E], F32, tag="scr")
                    for u in range(4):
                        s = t * 4 + u
                        nc.vector.tensor_tensor(
                            scratch[:],
                            l3t_ps[:, u * E : (u + 1) * E],
                            mask_sb[:, s * E : (s + 1) * E],
                            op=mybir.AluOpType.mult,
                        )
                        nc.vector.reduce_sum(
                            out_sb[:, s : s + 1],
                            scratch[:],
                            axis=mybir.AxisListType.X,
                        )
                nc.sync.dma_start(out_col[:], out_sb[:])

    if split:
        split_sync_waits(nc)
    return nc


# ---- host side ----

_cache = {}


def _get_nc(reps=1):
    key = ("nc", reps)
    if key not in _cache:
        _cache[key] = build_nc(reps)
    return _cache[key]


def _np_dtype():
    return ml_dtypes.bfloat16 if XDT == BF16 else np.float32


def prep_in_maps(
    expert_input, router_input, router_w, router_b,
    l1_w, l1_b, l1f_w, l1f_b, l2_w, l2_b, out_w, out_b,
):
    xdt = _np_dtype()
    f32 = np.float32

    # l1 merged weights, feature permutation:
    # new rows 0:60 -> (e=0..3) l1x feats, 60:64 -> l1x_out e0..3,
    #          64:124 -> (e=4..7) l1x feats, 124:128 -> l1x_out e4..7
    mw = (np.asarray(l1_w, f32) + np.tile(np.asarray(l1f_w, f32), (E, 1)))
    mb = (np.asarray(l1_b, f32) + np.tile(np.asarray(l1f_b, f32), E))
    perm = []
    for g in range(2):
        for i in range(4):
            e = 4 * g + i
            perm += [e * (L2 + 1) + o for o in range(L2)]
        perm += [(4 * g + i) * (L2 + 1) + L2 for i in range(4)]
    perm = np.array(perm)
    wT = np.ascontiguousarray(mw[perm].T).astype(xdt)          # [D, 128]
    bvec = mb[perm].reshape(128, 1).astype(f32)

    w2 = np.asarray(l2_w, f32).reshape(E, L3, 2 * L2)
    lq = np.zeros((128, 256), f32)
    lr = np.zeros((128, 256), f32)
    zbias = np.zeros((128, 2), f32)
    lo = np.zeros((128, 16), f32)
    sel = np.zeros((128, 16), f32)
    l2b = np.asarray(l2_b, f32).reshape(E, L3)
    ow = np.asarray(out_w, f32)
    for g in range(2):
        for i in range(4):
            e = 4 * g + i
            rowb = 64 * g + 15 * i
            colb = 128 * g + 32 * i
            lq[rowb : rowb + 15, colb : colb + 32] = w2[e, :, 0:L2].T
            lr[rowb : rowb + 15, colb : colb + 32] = w2[e, :, L2 : 2 * L2].T
            zbias[32 * i : 32 * i + 32, g] = l2b[e]
            lo[32 * i : 32 * i + 32, 8 * g + 4 * g + i] = ow[e]
            sel[64 * g + 60 + i, 8 * g + 4 * g + i] = 1.0
    obias = np.asarray(out_b, f32).reshape(8, 1)
    rwT = np.ascontiguousarray(np.asarray(router_w, f32).T)    # [R, E]
    rbrow = np.tile(np.asarray(router_b, f32).reshape(1, E), (1, NS))
    onesrow = np.ones((1, 128), f32)
    onescol = np.ones((128, 1), f32)

    shared = dict(
        wT=wT, bvec=bvec, lq=lq, lr=lr, zbias=zbias, lo=lo, sel=sel,
        obias=obias, rwT=rwT, rbrow=rbrow, onesrow=onesrow, onescol=onescol,
    )
    x = np.asarray(expert_input, f32)
    r = np.asarray(router_input, f32)
    in_maps = []
    for c in range(NCORES):
        sl = slice(c * BC, (c + 1) * BC)
        m = dict(shared)
        m["xT"] = np.ascontiguousarray(x[sl].T).astype(xdt)
        m["rT"] = np.ascontiguousarray(r[sl].T)
        in_maps.append(m)
    return in_maps


def postprocess(results):
    f64 = np.float64
    out_full = np.empty((B, 1), np.float32)
    masksum = np.zeros(E, f64)
    probsum = np.zeros(E, f64)
    zsum = entsum = topsum = 0.0
    for c, res in enumerate(results):
        oc = res["out_col"]                       # [128, NS]
        out_full[c * BC : (c + 1) * BC, 0] = oc.T.reshape(-1)
        st = res["stats"][0].astype(f64)          # [304]
        masksum += st[0:128].reshape(NS, E).sum(0)
        probsum += st[128:256].reshape(NS, E).sum(0)
        zsum += st[256 : 256 + NS].sum()
        entsum += st[272 : 272 + NS].sum()
        topsum += st[288 : 288 + NS].sum()
    frac = masksum / B
    avg = probsum / B
    aux = E * float((frac * avg).sum())
    z = zsum / B
    ent = entsum / B
    nent = ent / math.log(E)
    top1 = topsum / B
    rl = AUX_ALPHA * aux + Z_ALPHA * z
    f32 = np.float32
    return (
        out_full,
        f32(rl),
        f32(aux),
        f32(z),
        frac.astype(f32),
        avg.astype(f32),
        f32(nent),
        f32(top1),
    )


def kernel(**inputs):
    nc = _get_nc(reps=1)
    in_maps = prep_in_maps(**inputs)
    res = bass_utils.run_bass_kernel_spmd(
        nc, in_maps, core_ids=list(range(NCORES))
    )
    return postprocess(res.results)


# revision 11
# speedup vs baseline: 1.0593x; 1.0593x over previous
"""Trainium2 Bass kernel for nn_MoELayerStacks (moe_routing).

Computation (see reference):
  router:  gate_logits = router_input @ router_w.T + router_b  -> softmax,
           argmax one-hot, aux/z losses, entropy, top1 (batch means).
  l1:      [B,3072] @ [3072,128] (merged factorized weights), bias,
           split 15 l1x + 1 l1x_out per expert,
           h = clip([sq(l1x)*255/256, l1x], 0, 1).
  l2:      per-expert [30 -> 32], clip to [0,1].
  l3:      per-expert [32 -> 1] + l1x_out;  out[b] = l3x[b, argmax[b]].

Sharding: data-parallel over batch across 8 cores (2048 rows each), tiny
weights replicated; scalar/batch-mean reductions finished on host.

Device layout notes:
  - Host pre-transposes activations (xT [D, Bc], rT [R, Bc]) so the
    contraction dim lands on SBUF partitions; everything on-device runs in
    "feature-on-partition, batch-on-free" orientation until a final PE
    transpose of the tiny [8, B] l3x tile.
  - l1 output features are permuted so each expert-group's l2 inputs form a
    32-aligned partition window (matmul operand base partitions must be in
    {0, 32, 64, 96}): rows 0:60 = experts 0-3 l1x, 60:64 = l1x_out e0-3,
    64:124 = experts 4-7 l1x, 124:128 = l1x_out e4-7.
  - l2/l3 use block-diagonal weights (4 experts per matmul); the l1x_out
    residual is folded into the l3 PSUM accumulation via selector matmuls.
  - fp32 matmuls are bitcast to float32r (full PE rate at N>=256).
  - Router stats (mask/probs/z/entropy/top1 row values) are reduced over
    the batch with ones-vector matmuls into a [1, 304] stats tensor.
"""

import math
from contextlib import ExitStack

import numpy as np
import ml_dtypes

import concourse.bass as bass
import concourse.mybir as mybir
import concourse.tile as tile
from concourse import bass_utils
from concourse.masks import make_identity

# ---- problem constants (hardcoded per contract) ----
E = 8
L2 = 15
L3 = 32
D = 3072
R = 256
B = 16384
AUX_ALPHA = 0.01
Z_ALPHA = 0.001

NCORES = 8
BC = B // NCORES          # 2048 rows per core
TN = 512                  # batch tile (PSUM bank = 512 fp32)
NT = BC // TN             # 4 batch tiles per core
NS = BC // 128            # 16 subtiles of 128 rows
KC = D // 128             # 24 contraction chunks

F32 = mybir.dt.float32
F32R = mybir.dt.float32r
BF16 = mybir.dt.bfloat16

XDT = BF16                # dtype for x / l1 weights (DMA-dominant tensors)
SQ_SCALE = math.sqrt(255.0 / 256.0)


def _r(ap):
    """Bitcast an fp32 AP to float32r for full-rate PE streaming."""
    return ap.bitcast(F32R)


def split_sync_waits(nc, max_waits=1):
    """This walrus build caps sync waits per instruction (CTRL drain fails
    with >1); hoist extras onto preceding same-engine NoOps."""
    n = 0
    for f in nc.m.functions:
        for blk in f.blocks:
            new_insts = []
            for inst in blk.instructions:
                si = inst.sync_info
                if si is not None and si.on_wait and len(si.on_wait) > max_waits:
                    waits = list(si.on_wait)
                    extra, keep = waits[:-max_waits], waits[-max_waits:]
                    for i in range(0, len(extra), max_waits):
                        n += 1
                        nop = mybir.InstNoOp(name=f"W-split-{n}", ins=[], outs=[])
                        nop.engine = inst.engine
                        nop.sync_info = mybir.SyncInfo(
                            on_wait=extra[i : i + max_waits], on_update=[]
                        )
                        new_insts.append(nop)
                    inst.sync_info = mybir.SyncInfo(
                        on_wait=keep, on_update=list(si.on_update or [])
                    )
                new_insts.append(inst)
            blk.instructions = new_insts
    return n


def build_nc(reps: int = 1, split: bool = True) -> bass.Bass:
    nc = bass.Bass(target_bir_lowering=True)

    # ---- per-core DRAM tensors ----
    xT = nc.dram_tensor("xT", [D, BC], XDT, kind="ExternalInput")
    rT = nc.dram_tensor("rT", [R, BC], F32, kind="ExternalInput")
    wT = nc.dram_tensor("wT", [D, 128], XDT, kind="ExternalInput")
    bvec = nc.dram_tensor("bvec", [128, 1], F32, kind="ExternalInput")
    lq = nc.dram_tensor("lq", [128, 256], F32, kind="ExternalInput")
    lr = nc.dram_tensor("lr", [128, 256], F32, kind="ExternalInput")
    zbias = nc.dram_tensor("zbias", [128, 2], F32, kind="ExternalInput")
    lo = nc.dram_tensor("lo", [128, 16], F32, kind="ExternalInput")
    sel = nc.dram_tensor("sel", [128, 16], F32, kind="ExternalInput")
    obias = nc.dram_tensor("obias", [8, 1], F32, kind="ExternalInput")
    rwT = nc.dram_tensor("rwT", [R, E], F32, kind="ExternalInput")
    rbrow = nc.dram_tensor("rbrow", [1, NS * E], F32, kind="ExternalInput")
    onesrow = nc.dram_tensor("onesrow", [1, 128], F32, kind="ExternalInput")
    onescol = nc.dram_tensor("onescol", [128, 1], F32, kind="ExternalInput")

    out_col = nc.dram_tensor("out_col", [128, NS], F32, kind="ExternalOutput")
    stats = nc.dram_tensor("stats", [1, 304], F32, kind="ExternalOutput")

    with ExitStack() as ctx:
        tc = ctx.enter_context(tile.TileContext(nc))
        cpool = ctx.enter_context(tc.tile_pool(name="consts", bufs=1))

        # ---- resident constants ----
        wT_sb = cpool.tile([128, KC * 128], XDT)       # chunk k at cols k*128
        for k in range(KC):
            nc.sync.dma_start(
                wT_sb[:, k * 128 : (k + 1) * 128], wT[k * 128 : (k + 1) * 128, :]
            )
        rT_sb = cpool.tile([128, 2 * BC], F32)         # chunk k at cols k*BC
        for k in range(2):
            nc.sync.dma_start(
                rT_sb[:, k * BC : (k + 1) * BC], rT[k * 128 : (k + 1) * 128, :]
            )
        rw_sb = cpool.tile([128, 2 * E], F32)          # chunk k at cols k*8
        for k in range(2):
            nc.sync.dma_start(
                rw_sb[:, k * E : (k + 1) * E], rwT[k * 128 : (k + 1) * 128, :]
            )
        bvec_sb = cpool.tile([128, 1], F32)
        nc.sync.dma_start(bvec_sb[:], bvec[:])
        lq_sb = cpool.tile([128, 256], F32)
        nc.sync.dma_start(lq_sb[:], lq[:])
        lr_sb = cpool.tile([128, 256], F32)
        nc.sync.dma_start(lr_sb[:], lr[:])
        zb_sb = cpool.tile([128, 2], F32)
        nc.sync.dma_start(zb_sb[:], zbias[:])
        lo_sb = cpool.tile([128, 16], F32)
        nc.sync.dma_start(lo_sb[:], lo[:])
        sel_sb = cpool.tile([128, 16], F32)
        nc.sync.dma_start(sel_sb[:], sel[:])
        ob_sb = cpool.tile([8, 1], F32)
        nc.sync.dma_start(ob_sb[:], obias[:])
        rb_sb = cpool.tile([1, NS * E], F32)
        nc.sync.dma_start(rb_sb[:], rbrow[:])
        or_sb = cpool.tile([1, 128], F32)
        nc.sync.dma_start(or_sb[:], onesrow[:])
        oc_sb = cpool.tile([128, 1], F32)
        nc.sync.dma_start(oc_sb[:], onescol[:])
        ident = cpool.tile([128, 128], F32)
        make_identity(nc, ident[:])

        # f32r-rounded copies of the l2/l3 weights (walrus requires f32r
        # matmul inputs to be produced by a rounding instruction)
        lq_r = cpool.tile([128, 256], F32R)
        nc.vector.tensor_copy(lq_r[:], lq_sb[:])
        lr_r = cpool.tile([128, 256], F32R)
        nc.vector.tensor_copy(lr_r[:], lr_sb[:])
        lo_r = cpool.tile([128, 16], F32R)
        nc.vector.tensor_copy(lo_r[:], lo_sb[:])
        sel_r = cpool.tile([128, 16], F32R)
        nc.vector.tensor_copy(sel_r[:], sel_sb[:])

        # persistent work tiles (whole-core router state, output staging)
        mask_sb = cpool.tile([128, NS * E], F32)
        out_sb = cpool.tile([128, NS], F32)

        xpool = ctx.enter_context(tc.tile_pool(name="xp", bufs=3))
        rpool = ctx.enter_context(tc.tile_pool(name="rtr", bufs=2))
        wpool = ctx.enter_context(tc.tile_pool(name="work", bufs=2))
        ps_r = ctx.enter_context(tc.tile_pool(name="ps_r", bufs=1, space="PSUM"))
        ps1 = ctx.enter_context(tc.tile_pool(name="ps1", bufs=2, space="PSUM"))
        psz = ctx.enter_context(tc.tile_pool(name="psz", bufs=2, space="PSUM"))
        ps3 = ctx.enter_context(tc.tile_pool(name="ps3", bufs=1, space="PSUM"))
        pst = ctx.enter_context(tc.tile_pool(name="pst", bufs=1, space="PSUM"))

        for _ in range(reps):
            if True:
                # issue first x slab load before router work so it overlaps
                x_sb = xpool.tile([128, KC * TN], XDT, tag="xslab")
                nc.sync.dma_start(
                    x_sb[:].rearrange("p (k n) -> p k n", n=TN),
                    xT[:]
                    .rearrange("(k p) b -> p k b", p=128)[:, :, 0:TN],
                )
                slabs = [x_sb]

                # ---- phase 1: router for the whole core ----
                gl_ps = ps_r.tile([128, NS * E], F32, tag="gl")
                for s in range(NS):
                    for k in range(2):
                        nc.tensor.matmul(
                            gl_ps[:, s * E : (s + 1) * E],
                            rT_sb[:, k * BC + s * 128 : k * BC + (s + 1) * 128],
                            rw_sb[:, k * E : (k + 1) * E],
                            start=(k == 0),
                            stop=False,
                        )
                    nc.tensor.matmul(
                        gl_ps[:, s * E : (s + 1) * E],
                        or_sb[0:1, :],
                        rb_sb[0:1, s * E : (s + 1) * E],
                        start=False,
                        stop=True,
                    )
                gl_sb = rpool.tile([128, NS * E], F32)
                nc.scalar.copy(gl_sb[:], gl_ps[:])
                gl3 = gl_sb[:].rearrange("p (s e) -> p s e", e=E)

                m_sb = rpool.tile([128, NS], F32)
                nc.vector.reduce_max(m_sb[:], gl3, axis=mybir.AxisListType.X)
                mb = m_sb[:, :, None].broadcast_to([128, NS, E])

                t_sb = rpool.tile([128, NS * E], F32)
                t3 = t_sb[:].rearrange("p (s e) -> p s e", e=E)
                nc.vector.tensor_tensor(t3, gl3, mb, op=mybir.AluOpType.subtract)

                e_sb = rpool.tile([128, NS * E], F32)
                nc.scalar.activation(e_sb[:], t_sb[:], mybir.ActivationFunctionType.Exp)
                e3 = e_sb[:].rearrange("p (s e) -> p s e", e=E)

                ssum = rpool.tile([128, NS], F32)
                nc.vector.reduce_sum(ssum[:], e3, axis=mybir.AxisListType.X)
                rec = rpool.tile([128, NS], F32)
                nc.vector.reciprocal(rec[:], ssum[:])
                recb = rec[:, :, None].broadcast_to([128, NS, E])

                probs = rpool.tile([128, NS * E], F32)
                p3 = probs[:].rearrange("p (s e) -> p s e", e=E)
                nc.vector.tensor_tensor(p3, e3, recb, op=mybir.AluOpType.mult)

                m3 = mask_sb[:].rearrange("p (s e) -> p s e", e=E)
                nc.vector.tensor_tensor(m3, gl3, mb, op=mybir.AluOpType.is_equal)

                lns = rpool.tile([128, NS], F32)
                nc.scalar.activation(lns[:], ssum[:], mybir.ActivationFunctionType.Ln)
                lse = rpool.tile([128, NS], F32)
                nc.vector.tensor_add(lse[:], m_sb[:], lns[:])
                zrow = rpool.tile([128, NS], F32)
                nc.vector.tensor_mul(zrow[:], lse[:], lse[:])

                et_sb = rpool.tile([128, NS * E], F32)
                et3 = et_sb[:].rearrange("p (s e) -> p s e", e=E)
                nc.vector.tensor_tensor(et3, e3, t3, op=mybir.AluOpType.mult)
                u_sb = rpool.tile([128, NS], F32)
                nc.vector.reduce_sum(u_sb[:], et3, axis=mybir.AxisListType.X)
                v_sb = rpool.tile([128, NS], F32)
                nc.vector.tensor_mul(v_sb[:], u_sb[:], rec[:])
                entrow = rpool.tile([128, NS], F32)
                nc.vector.tensor_sub(entrow[:], lns[:], v_sb[:])

                stats_ps = ps_r.tile([1, 304], F32, tag="stats")
                for rhs, off, w in (
                    (mask_sb, 0, 128),
                    (probs, 128, 128),
                    (zrow, 256, NS),
                    (entrow, 272, NS),
                    (rec, 288, NS),
                ):
                    nc.tensor.matmul(
                        stats_ps[0:1, off : off + w],
                        oc_sb[:, 0:1],
                        rhs[:, 0:w],
                        start=True,
                        stop=True,
                    )
                stats_sb = rpool.tile([1, 304], F32)
                nc.scalar.copy(stats_sb[:], stats_ps[:])
                nc.sync.dma_start(stats[:], stats_sb[:])

                # ---- phase 2: l1/l2/l3 over 4 batch tiles ----
                for t in range(NT):
                    if t + 1 < NT:
                        x_nx = xpool.tile([128, KC * TN], XDT, tag="xslab")
                        nc.sync.dma_start(
                            x_nx[:].rearrange("p (k n) -> p k n", n=TN),
                            xT[:]
                            .rearrange("(k p) b -> p k b", p=128)[
                                :, :, (t + 1) * TN : (t + 2) * TN
                            ],
                        )
                        slabs.append(x_nx)
                    x_cur = slabs[t]

                    l1_ps = ps1.tile([128, TN], F32, tag="l1")
                    for k in range(KC):
                        nc.tensor.matmul(
                            l1_ps[:],
                            wT_sb[:, k * 128 : (k + 1) * 128],
                            x_cur[:, k * TN : (k + 1) * TN],
                            start=(k == 0),
                            stop=(k == KC - 1),
                        )
                    l1c = wpool.tile([128, TN], F32R, tag="l1c")
                    nc.scalar.activation(
                        l1c[:],
                        l1_ps[:],
                        mybir.ActivationFunctionType.Identity,
                        bias=bvec_sb[:, 0:1],
                    )
                    # h = [clip(sq(l1x)*255/256, 0, 1), clip(l1x, 0, 1)]
                    # (rows 60:64 / 124:128 are l1x_out; squared junk unused)
                    q_sb = wpool.tile([124, TN], F32R, tag="q")
                    nc.scalar.activation(
                        q_sb[0:124, :],
                        l1c[0:124, :],
                        mybir.ActivationFunctionType.Square,
                        scale=SQ_SCALE,
                    )
                    nc.vector.tensor_scalar_min(q_sb[0:124, :], q_sb[0:124, :], 1.0)
                    r_sb = wpool.tile([124, TN], F32R, tag="r")
                    nc.vector.tensor_scalar(
                        r_sb[0:124, :],
                        l1c[0:124, :],
                        0.0,
                        1.0,
                        op0=mybir.AluOpType.max,
                        op1=mybir.AluOpType.min,
                    )

                    l3_ps = ps3.tile([8, TN], F32, tag="l3")
                    for g in range(2):
                        base = 64 * g
                        z_ps = psz.tile([128, TN], F32, tag="z")
                        nc.tensor.matmul(
                            z_ps[:],
                            lq_r[base : base + 60, g * 128 : (g + 1) * 128],
                            q_sb[base : base + 60, :],
                            start=True,
                            stop=False,
                        )
                        nc.tensor.matmul(
                            z_ps[:],
                            lr_r[base : base + 60, g * 128 : (g + 1) * 128],
                            r_sb[base : base + 60, :],
                            start=False,
                            stop=True,
                        )
                        z_sb = wpool.tile([128, TN], F32R, tag="zsb")
                        nc.scalar.activation(
                            z_sb[:],
                            z_ps[:],
                            mybir.ActivationFunctionType.Identity,
                            bias=zb_sb[:, g : g + 1],
                        )
                        nc.vector.tensor_scalar(
                            z_sb[:],
                            z_sb[:],
                            0.0,
                            1.0,
                            op0=mybir.AluOpType.max,
                            op1=mybir.AluOpType.min,
                        )
                        nc.tensor.matmul(
                            l3_ps[:],
                            lo_r[:, g * 8 : (g + 1) * 8],
                            z_sb[:],
                            start=(g == 0),
                            stop=False,
                        )
                        # fold l1x_out residual: selector over 32-aligned window
                        nc.tensor.matmul(
                            l3_ps[:],
                            sel_r[32 + 64 * g : 64 + 64 * g, g * 8 : (g + 1) * 8],
                            l1c[32 + 64 * g : 64 + 64 * g, :],
                            start=False,
                            stop=(g == 1),
                            tile_position=(32 + 64 * g, 0),
                        )
                    l3x = wpool.tile([8, TN], F32, tag="l3x")
                    nc.scalar.activation(
                        l3x[:],
                        l3_ps[:],
                        mybir.ActivationFunctionType.Identity,
                        bias=ob_sb[:, 0:1],
                    )
                    l3t_ps = pst.tile([128, 4 * E], F32, tag="l3t")
                    for u in range(4):
                        nc.tensor.transpose(
                            l3t_ps[:, u * E : (u + 1) * E],
                            l3x[:, u * 128 : (u + 1) * 128],
                            ident[0:8, 0:8],
                        )
                    scratch = wpool.tile([128, E], F32, tag="scr")
                    for u in range(4):
                        s = t * 4 + u
                        nc.vector.tensor_tensor(
                            scratch[:],
                            l3t_ps[:, u * E : (u + 1) * E],
                            mask_sb[:, s * E : (s + 1) * E],
                            op=mybir.AluOpType.mult,
                        )
                        nc.vector.reduce_sum(
                            out_sb[:, s : s + 1],
                            scratch[:],
                            axis=mybir.AxisListType.X,
                        )
                nc.sync.dma_start(out_col[:], out_sb[:])

    if split:
        split_sync_waits(nc)
    return nc


# ---- host side ----

_cache = {}


def _get_nc(reps=1):
    key = ("nc", reps)
    if key not in _cache:
        _cache[key] = build_nc(reps)
    return _cache[key]


def _np_dtype():
    return ml_dtypes.bfloat16 if XDT == BF16 else np.float32


def prep_in_maps(
    expert_input, router_input, router_w, router_b,
    l1_w, l1_b, l1f_w, l1f_b, l2_w, l2_b, out_w, out_b,
):
    xdt = _np_dtype()
    f32 = np.float32

    # l1 merged weights, feature permutation:
    # new rows 0:60 -> (e=0..3) l1x feats, 60:64 -> l1x_out e0..3,
    #          64:124 -> (e=4..7) l1x feats, 124:128 -> l1x_out e4..7
    mw = (np.asarray(l1_w, f32) + np.tile(np.asarray(l1f_w, f32), (E, 1)))
    mb = (np.asarray(l1_b, f32) + np.tile(np.asarray(l1f_b, f32), E))
    perm = []
    for g in range(2):
        for i in range(4):
            e = 4 * g + i
            perm += [e * (L2 + 1) + o for o in range(L2)]
        perm += [(4 * g + i) * (L2 + 1) + L2 for i in range(4)]
    perm = np.array(perm)
    wT = np.ascontiguousarray(mw[perm].T).astype(xdt)          # [D, 128]
    bvec = mb[perm].reshape(128, 1).astype(f32)

    w2 = np.asarray(l2_w, f32).reshape(E, L3, 2 * L2)
    lq = np.zeros((128, 256), f32)
    lr = np.zeros((128, 256), f32)
    zbias = np.zeros((128, 2), f32)
    lo = np.zeros((128, 16), f32)
    sel = np.zeros((128, 16), f32)
    l2b = np.asarray(l2_b, f32).reshape(E, L3)
    ow = np.asarray(out_w, f32)
    for g in range(2):
        for i in range(4):
            e = 4 * g + i
            rowb = 64 * g + 15 * i
            colb = 128 * g + 32 * i
            lq[rowb : rowb + 15, colb : colb + 32] = w2[e, :, 0:L2].T
            lr[rowb : rowb + 15, colb : colb + 32] = w2[e, :, L2 : 2 * L2].T
            zbias[32 * i : 32 * i + 32, g] = l2b[e]
            lo[32 * i : 32 * i + 32, 8 * g + 4 * g + i] = ow[e]
            sel[64 * g + 60 + i, 8 * g + 4 * g + i] = 1.0
    obias = np.asarray(out_b, f32).reshape(8, 1)
    rwT = np.ascontiguousarray(np.asarray(router_w, f32).T)    # [R, E]
    rbrow = np.tile(np.asarray(router_b, f32).reshape(1, E), (1, NS))
    onesrow = np.ones((1, 128), f32)
    onescol = np.ones((128, 1), f32)

    shared = dict(
        wT=wT, bvec=bvec, lq=lq, lr=lr, zbias=zbias, lo=lo, sel=sel,
        obias=obias, rwT=rwT, rbrow=rbrow, onesrow=onesrow, onescol=onescol,
    )
    x = np.asarray(expert_input, f32)
    r = np.asarray(router_input, f32)
    in_maps = []
    for c in range(NCORES):
        sl = slice(c * BC, (c + 1) * BC)
        m = dict(shared)
        m["xT"] = np.ascontiguousarray(x[sl].T).astype(xdt)
        m["rT"] = np.ascontiguousarray(r[sl].T)
        in_maps.append(m)
    return in_maps


def postprocess(results):
    f64 = np.float64
    out_full = np.empty((B, 1), np.float32)
    masksum = np.zeros(E, f64)
    probsum = np.zeros(E, f64)
    zsum = entsum = topsum = 0.0
    for c, res in enumerate(results):
        oc = res["out_col"]                       # [128, NS]
        out_full[c * BC : (c + 1) * BC, 0] = oc.T.reshape(-1)
        st = res["stats"][0].astype(f64)          # [304]
        masksum += st[0:128].reshape(NS, E).sum(0)
        probsum += st[128:256].reshape(NS, E).sum(0)
        zsum += st[256 : 256 + NS].sum()
        entsum += st[272 : 272 + NS].sum()
        topsum += st[288 : 288 + NS].sum()
    frac = masksum / B
    avg = probsum / B
    aux = E * float((frac * avg).sum())
    z = zsum / B
    ent = entsum / B
    nent = ent / math.log(E)
    top1 = topsum / B
    rl = AUX_ALPHA * aux + Z_ALPHA * z
    f32 = np.float32
    return (
        out_full,
        f32(rl),
        f32(aux),
        f32(z),
        frac.astype(f32),
        avg.astype(f32),
        f32(nent),
        f32(top1),
    )


def kernel(**inputs):
    nc = _get_nc(reps=1)
    in_maps = prep_in_maps(**inputs)
    res = bass_utils.run_bass_kernel_spmd(
        nc, in_maps, core_ids=list(range(NCORES))
    )
    return postprocess(res.results)
